# revision 12
# baseline (speedup 1.0000x reference)
"""Fused OOQKV attention-with-generated-transform kernel for Trainium2.

Math (per head h):
  g = gelu(x @ Wg_h + bg_h)            # [T, 64, 64] per-token transform
  q,k,v = x @ W{q,k,v}_h + b           # [T, 64]
  qg[t] = q[t] @ g[t]
  att = softmax(qg @ k^T)              # per batch, no scaling
  out_h = att @ v

Sharding: head-parallel, 1 head per core (8 heads, 8 cores); every core
reads the full (host-pre-transposed) xT.

Per-core schedule:
  phase 1 (per 128-token tile): fused q|v_aug|k projection and the
    32768-wide g projection, grouped so consecutive PE matmuls share the
    stationary xT slice (f32r weight switches cost ~2x); biases are K=1
    bf16 matmuls (bf16 keeps them at stream rate; bias magnitudes are
    ~0.04 so bf16 rounding is ~1e-4 absolute). ACT applies exact gelu,
    writing each 512-chunk transposed to (e-major, d-minor) layout so the
    DVE qg contraction multiplies contiguously against a broadcast q view
    and reduces over a contiguous innermost d. PE transposes build kT and
    qgT for phase 2.
  phase 2 (per batch, per 512 query cols): S^T = kT-slice.T @ qgT on PE,
    exp on ACT (no max subtraction; |scores| < 70 so fp32 exp is exact
    enough), then out^T accumulated over m-tiles with v augmented by a
    ones column so row 64 carries the softmax denominator.
Host divides by the denominator row and transposes during the gather.

Matmuls run in float32r (fp32-reduced: 1 cycle/row streaming, ~1e-4
matmul rel err measured on HW); end-to-end rel err vs the fp32 reference
is ~1e-3.
"""

import sys

sys.path.insert(0, "/opt/trn_rl_repo")

import numpy as np

B, N, E, H, D = 4, 1024, 512, 8, 64
T = B * N                 # 4096 flattened tokens
OC = 512                  # g-matmul output chunk
NOC = (D * D) // OC       # 8 chunks per head
DPC = OC // D             # 8 d-values per chunk
NTT = T // 128            # 32 token tiles
NKT = E // 128            # 4 contraction tiles
QVKW = 256                # fused q|v_aug|k projection width (zero padded)
M = 8                     # cores

_cache = {}


def _build():
    if "nc" in _cache:
        return _cache["nc"]
    from contextlib import ExitStack

    import concourse.bass as bass
    import concourse.bacc as bacc
    import concourse.mybir as mybir
    import concourse.tile as tile
    from concourse.masks import make_identity

    F32 = mybir.dt.float32
    F32R = mybir.dt.float32r
    BF16 = mybir.dt.bfloat16
    AF = mybir.ActivationFunctionType
    ALU = mybir.AluOpType
    AX = mybir.AxisListType

    F8 = mybir.dt.float8e4
    DR = mybir.MatmulPerfMode.DoubleRow

    nc = bacc.Bacc(trn_type="TRN2")
    xT_d = nc.dram_tensor("xT", [E, T], F32R, kind="ExternalInput")
    Wg_d = nc.dram_tensor("Wg", [E, D * D], F32R, kind="ExternalInput")
    bg_d = nc.dram_tensor("bg", [1, 2 * D * D], F8, kind="ExternalInput")
    Wqvk_d = nc.dram_tensor("Wqvk", [E, QVKW], F32R, kind="ExternalInput")
    bqvk_d = nc.dram_tensor("bqvk", [1, 2 * QVKW], F8, kind="ExternalInput")
    outT_d = nc.dram_tensor("outT", [D + 1, T], F32, kind="ExternalOutput")

    with tile.TileContext(nc) as tc, ExitStack() as ctx:
        const = ctx.enter_context(tc.tile_pool(name="const", bufs=1))
        acts = ctx.enter_context(tc.tile_pool(name="acts", bufs=1))

        wqvk_sb = []
        for kt in range(NKT):
            wqt = const.tile([128, QVKW], F32R, tag=f"wqvk{kt}")
            nc.sync.dma_start(wqt[:], Wqvk_d[kt * 128:(kt + 1) * 128, :])
            wqvk_sb.append(wqt)
        bg_sb = const.tile([1, 2 * D * D], F8)
        nc.sync.dma_start(bg_sb[:], bg_d[:, :])
        bqvk_sb = const.tile([1, 2 * QVKW], F8)
        nc.sync.dma_start(bqvk_sb[:], bqvk_d[:, :])
        # DoubleRow bias lhsT: row 0 weight 1.0 (hi bias), row 1 weight
        # 1/16 (scaled residual) -> bias applied at ~fp16 accuracy for half
        # the PE cost of a bf16 K=1 matmul.
        ones2 = const.tile([1, 256], F8)
        nc.gpsimd.memset(ones2[:, 0:128], 1.0)
        nc.gpsimd.memset(ones2[:, 128:256], 0.0625)
        o2 = ones2[:]
        ones2_v = bass.AP(tensor=o2.tensor, offset=o2.offset,
                          ap=[o2.ap[0], [128, 2], [1, 128]])
        ident = const.tile([128, 128], F32)
        make_identity(nc, ident[:])

        def bias2_v(sb, half, off, width):
            # view of [hi | lo] packed bias: rows (hi@off, lo@half+off)
            w = sb[:]
            return bass.AP(tensor=w.tensor, offset=w.offset + off,
                           ap=[w.ap[0], [half, 2], [1, width]])

        # persistent per-head activations
        q_sb = acts.tile([128, NTT, D], F32)       # q, natural layout
        v_sb = acts.tile([128, NTT, D + 1], F32R)  # v | ones column
        kT_sb = acts.tile([D, T], F32R)
        qgT_sb = acts.tile([D, T], F32R)

        # ---------------- phase 1: projections, g, qg ----------------
        with ExitStack() as p1:
            xpool = p1.enter_context(tc.tile_pool(name="xp", bufs=2))
            wgpool = p1.enter_context(tc.tile_pool(name="wgp", bufs=1))
            wg_sb = []
            for kt in range(NKT):
                wgt = wgpool.tile([128, D * D], F32R, tag=f"wg{kt}",
                                  name=f"wg{kt}")
                wg_sb.append(wgt)
            QL = (D * D) // 4
            for quar in range(4):
                for kt in range(NKT):
                    nc.scalar.dma_start(
                        wg_sb[kt][:, quar * QL:(quar + 1) * QL],
                        Wg_d[kt * 128:(kt + 1) * 128,
                             quar * QL:(quar + 1) * QL])
            gpool = p1.enter_context(tc.tile_pool(name="gp", bufs=5))
            dpool = p1.enter_context(tc.tile_pool(name="dp", bufs=4))
            pp_g = p1.enter_context(
                tc.tile_pool(name="pg", bufs=7, space="PSUM"))
            pp_qvk = pp_g
            pp_tr = p1.enter_context(
                tc.tile_pool(name="ptr", bufs=1, space="PSUM"))

            pending = []  # (tc0, k_nat, qg_t) awaiting PE transpose

            def flush_pending():
                for ptc0, pk, pqg in pending:
                    ptr = pp_tr.tile([D, 128], F32, tag="tr", name="ktr")
                    nc.tensor.transpose(ptr[:], pk[:], ident[:])
                    nc.vector.tensor_copy(kT_sb[:, ptc0:ptc0 + 128], ptr[:])
                    ptr2 = pp_tr.tile([D, 128], F32, tag="tr", name="qgtr")
                    nc.tensor.transpose(ptr2[:], pqg[:], ident[:])
                    nc.vector.tensor_copy(qgT_sb[:, ptc0:ptc0 + 128], ptr2[:])
                pending.clear()

            for tt in range(NTT):
                tc0 = tt * 128
                xs = []
                for kt in range(NKT):
                    xt = xpool.tile([128, 128], F32R, tag=f"x{kt}")
                    nc.sync.dma_start(
                        xt[:], xT_d[kt * 128:(kt + 1) * 128, tc0:tc0 + 128])
                    xs.append(xt)

                # two rounds of 4 g-chunks; round 0 also carries the qvk
                # projection so each (round, kt) is a same-lhsT matmul run
                pq = pp_qvk.tile([128, OC], F32, tag="pg", name="pq")
                pgs = {}
                for rnd in range(2):
                    for kt in range(NKT):
                        if rnd == 0:
                            nc.tensor.matmul(pq[:, 0:QVKW], xs[kt][:],
                                             wqvk_sb[kt][:],
                                             start=(kt == 0), stop=False)
                        for oc in range(rnd * 4, rnd * 4 + 4):
                            oc0 = oc * OC
                            if kt == 0:
                                pgs[oc] = pp_g.tile([128, OC], F32, tag="pg", name=f"pg{oc}")
                            nc.tensor.matmul(
                                pgs[oc][:], xs[kt][:],
                                wg_sb[kt][:, oc0:oc0 + OC],
                                start=(kt == 0), stop=False)
                    if rnd == 0:
                        nc.tensor.matmul(pq[:, 0:QVKW], ones2_v,
                                         bias2_v(bqvk_sb, QVKW, 0, QVKW),
                                         start=False, stop=True, perf_mode=DR)
                    for oc in range(rnd * 4, rnd * 4 + 4):
                        oc0 = oc * OC
                        nc.tensor.matmul(pgs[oc][:], ones2_v,
                                         bias2_v(bg_sb, D * D, oc0, OC),
                                         start=False, stop=True, perf_mode=DR)
                    if rnd == 0:
                        flush_pending()  # prior tile's transposes mid-stream

                nc.vector.tensor_copy(q_sb[:, tt, :], pq[:, 0:D])
                nc.vector.tensor_copy(v_sb[:, tt, :], pq[:, D:2 * D + 1])
                k_nat = dpool.tile([128, D], F32, tag="knat")
                nc.vector.tensor_copy(k_nat[:], pq[:, 2 * D + 1:3 * D + 1])

                # gelu + qg contraction per chunk
                qg_part = dpool.tile([128, NOC, D], F32, tag="qgp")
                for oc in range(NOC):
                    # gelu written contiguously (d-major, same layout as
                    # psum); the e/d stride swap happens in the DVE multiply
                    # whose output is (e-major, d-minor) so the d-reduce is
                    # contiguous
                    g_t = gpool.tile([128, OC], F32, tag="g")
                    nc.scalar.activation(g_t[:], pgs[oc][:], AF.Gelu)
                    gw = g_t[:]
                    g_dmaj = bass.AP(tensor=gw.tensor, offset=gw.offset,
                                     ap=[gw.ap[0], [1, D], [D, DPC]])
                    prod = gpool.tile([128, OC], F32, tag="prod")
                    qs = q_sb[:, tt, :]
                    q3 = bass.AP(
                        tensor=qs.tensor,
                        offset=qs.offset + oc * DPC,
                        ap=[qs.ap[0], [0, D], [1, DPC]])
                    nc.vector.tensor_tensor(
                        prod[:].rearrange("p (e d) -> p e d", d=DPC),
                        g_dmaj,
                        q3, op=ALU.mult)
                    nc.vector.tensor_reduce(
                        qg_part[:, oc, :],
                        prod[:].rearrange("p (e d) -> p e d", d=DPC),
                        axis=AX.X, op=ALU.add)
                qg_t = dpool.tile([128, D], F32, tag="qg")
                qp = qg_part[:]
                qpv = bass.AP(tensor=qp.tensor, offset=qp.offset,
                              ap=[qp.ap[0], [1, D], [D, NOC]])
                nc.vector.tensor_reduce(qg_t[:], qpv, axis=AX.X, op=ALU.add)
                pending.append((tc0, k_nat, qg_t))
            flush_pending()

        # ---------------- phase 2: attention ----------------
        with ExitStack() as p2:
            espool = p2.enter_context(tc.tile_pool(name="es", bufs=34))
            outp = p2.enter_context(tc.tile_pool(name="outp", bufs=4))
            pp_s = p2.enter_context(
                tc.tile_pool(name="psc", bufs=6, space="PSUM"))
            pp_av = p2.enter_context(
                tc.tile_pool(name="pav", bufs=2, space="PSUM"))

            NMT = N // 128  # m tiles per batch
            NNC = N // OC   # n chunks per batch
            pending_av = []  # (b, nch, es-dict) awaiting av emission

            def emit_av():
                if not pending_av:
                    return
                by_b = {}
                for bb, nch, esd in pending_av:
                    by_b.setdefault(bb, {})[nch] = esd
                for bb, chunks in by_b.items():
                    pavs = {nch: pp_av.tile([D + 1, OC], F32, tag="av",
                                            name=f"pav{nch}")
                            for nch in chunks}
                    for mt in range(NMT):
                        for nch, esd in chunks.items():
                            nc.tensor.matmul(pavs[nch][:],
                                             v_sb[:, bb * NMT + mt, :],
                                             esd[mt][:],
                                             start=(mt == 0),
                                             stop=(mt == NMT - 1))
                    for nch in chunks:
                        nc0 = bb * N + nch * OC
                        o_t = outp.tile([D + 1, OC], F32, tag="o", name="o_t")
                        nc.vector.tensor_copy(o_t[:], pavs[nch][:])
                        nc.sync.dma_start(outT_d[:, nc0:nc0 + OC], o_t[:])
                pending_av.clear()

            for b in range(B):
                es = {}
                # S^T and exp for the whole batch; kT slice (lhsT) is
                # reused across both n-chunks
                for mt in range(NMT):
                    if mt == 2:
                        emit_av()  # prior batch's av, mid-stream
                    mc0 = b * N + mt * 128
                    for nch in range(NNC):
                        nc0 = b * N + nch * OC
                        ps_ = pp_s.tile([128, OC], F32, tag="s")
                        nc.tensor.matmul(ps_[:], kT_sb[:, mc0:mc0 + 128],
                                         qgT_sb[:, nc0:nc0 + OC],
                                         start=True, stop=True)
                        e_t = espool.tile([128, OC], F32R, tag="es")
                        nc.scalar.activation(e_t[:], ps_[:], AF.Exp)
                        es[(mt, nch)] = e_t
                for nch in range(NNC):
                    pending_av.append((b, nch, {mt: es[(mt, nch)]
                                                for mt in range(NMT)}))
            emit_av()

    nc.compile()
    _cache["nc"] = nc
    return nc


def _fp8_hilo(b):
    """Pack a fp32 bias row [n] into [1, 2n] fp8e4m3 (hi | residual*16)."""
    import ml_dtypes
    f8 = ml_dtypes.float8_e4m3
    b = np.asarray(b, np.float32).reshape(-1)
    hi = b.astype(f8)
    lo = ((b - hi.astype(np.float32)) * 16.0).astype(f8)
    return np.concatenate([hi, lo]).reshape(1, 2 * b.shape[0])


def _make_in_maps(x, Wq, bq, Wk, bk, Wv, bv, Wg, bg):
    x = np.asarray(x, dtype=np.float32)
    xT = np.ascontiguousarray(x.reshape(T, E).T)
    in_maps = []
    for h in range(M):
        c0 = h * D
        Wqvk = np.zeros((E, QVKW), dtype=np.float32)
        Wqvk[:, 0:D] = Wq[:, c0:c0 + D]
        Wqvk[:, D:2 * D] = Wv[:, c0:c0 + D]
        # column 2*D is the ones column of v_aug: weight 0, bias 1
        Wqvk[:, 2 * D + 1:3 * D + 1] = Wk[:, c0:c0 + D]
        bqvk = np.zeros((1, QVKW), dtype=np.float32)
        bqvk[0, 0:D] = bq[c0:c0 + D]
        bqvk[0, D:2 * D] = bv[c0:c0 + D]
        bqvk[0, 2 * D] = 1.0
        bqvk[0, 2 * D + 1:3 * D + 1] = bk[c0:c0 + D]
        g0 = h * D * D
        in_maps.append(dict(
            xT=xT,
            Wg=np.ascontiguousarray(Wg[:, g0:g0 + D * D], dtype=np.float32),
            bg=_fp8_hilo(bg[g0:g0 + D * D]),
            Wqvk=Wqvk,
            bqvk=_fp8_hilo(bqvk),
        ))
    return in_maps


def kernel(x, Wq, bq, Wk, bk, Wv, bv, Wg, bg):
    from concourse import bass_utils

    nc = _build()
    in_maps = _make_in_maps(x, Wq, bq, Wk, bk, Wv, bv, Wg, bg)
    global _last_in_maps
    _last_in_maps = in_maps
    res = bass_utils.run_bass_kernel_spmd(nc, in_maps, core_ids=list(range(M)))
    out = np.empty((B, N, H, D), dtype=np.float32)
    for h in range(M):
        oT = res.results[h]["outT"]           # [65, T]
        o = (oT[:D] / oT[D:D + 1]).T          # [T, 64]
        out[:, :, h, :] = o.reshape(B, N, D)
    return out.reshape(B, N, E)



# revision 35
# speedup vs baseline: 1.0501x; 1.0501x over previous
"""Fused OOQKV attention kernel for Trainium2 — transposed-g design.

Math (per head h, one head per core):
  g = gelu(x @ Wg_h + bg_h)            # [T, 64, 64] per-token transform
  q,k,v = x @ W{q,k,v}_h + b           # [T, 64]
  qg[t] = q[t] @ g[t]
  att = softmax(qg @ k^T)              # per batch, no scaling
  out_h = att @ v

g is computed TRANSPOSED: outputs on partitions, tokens on the free
axis. Chunk oc (0..31) holds the 128 outputs {(d, e): e in {2oc, 2oc+1}}
at partition p = (e - 2oc)*64 + d (host permutes Wg's columns). This
kills the three taxes the natural layout paid:
  - bg becomes a per-partition scalar fused into the ACT gelu (K=1 bias
    matmuls cost ~300ns each / ~86us of PE regardless of dtype)
  - qg[e] = sum_d q_d g[(d,e)] becomes a partition-group reduction: PE
    indicator matmuls (~35us) instead of ~184us of DVE reduce. All 32
    indicators are column-shifted views of one [128, 127] tile
    (mega[p, j] = 1 iff j == 62 + p//64).
  - gelu, multiply and reduce all touch contiguous memory; the natural
    layout forces an (e,d) interleave tax on exactly one engine
    (measured: gelu 1149 vs 683ns, DVE mult 967 vs 602ns per 512 elems)
qgT and kT come out pre-transposed for attention; only v transposes
back to natural (av's lhsT contracts over tokens).

q and k are each projected DOUBLED ([W|W] -> 128 rows) so partition p
reads q at d = p%64 for the DVE multiply, and so pair-1 scores read
kT/qgT at matching base-64 partitions (no cross-partition copies,
which no compute engine can do).

Schedule: 2 blocks of 4 token-chunks (2048 tokens = 2 batches each).
Wg[oc, kt] stays stationary across the block's 4 N=512 streams.
Software pipelining: gelu+mult of chunk oc issue one round late, the
indicator matmuls two rounds late, so the PE queue never head-of-line
blocks on the ACT->DVE chain. Attention for a block's two batches
interleaves into the next block's PE stream; batches 2,3 drain in a
tail. PSUM (16KB/partition, 2KB banks): 4-buf g ring + [128, 1024] qg
accumulator (pair0 rows 0:64, pair1 rows 64:128, each matmul writing a
2KB bank window) + 2-buf attention/qkv ring = exactly 8 banks.

All matmuls dtype-homogeneous (compiler rejects 32/16-bit mixing):
f32r except av in bf16 (v, es rounded to bf16: ~1e-3 output error at
2e-2 tolerance). The softmax denominator rides as a memset ones-column
of v; the host divides and transposes during the gather.
"""

import sys

sys.path.insert(0, "/opt/trn_rl_repo")

import numpy as np

B, N, E, H, D = 4, 1024, 512, 8, 64
T = B * N                 # 4096 flattened tokens
NKT = E // 128            # 4 contraction tiles
NOC = (D * D) // 128      # 32 transposed g chunks
TCW = 512                 # token chunk width
NTC = T // TCW            # 8 token chunks
VW = D + 1                # v | ones column
M = 8                     # cores

_cache = {}


def _build():
    if "nc" in _cache:
        return _cache["nc"]
    from contextlib import ExitStack

    import concourse.bass as bass
    import concourse.bacc as bacc
    import concourse.mybir as mybir
    import concourse.tile as tile
    from concourse.masks import make_identity

    F32 = mybir.dt.float32
    F32R = mybir.dt.float32r
    BF16 = mybir.dt.bfloat16
    AF = mybir.ActivationFunctionType
    ALU = mybir.AluOpType

    nc = bacc.Bacc(trn_type="TRN2")
    xT_d = nc.dram_tensor("xT", [E, T], F32R, kind="ExternalInput")
    WgT_d = nc.dram_tensor("WgT", [E, D * D], F32R, kind="ExternalInput")
    bgT_d = nc.dram_tensor("bgT", [128, NOC], F32, kind="ExternalInput")
    Wqq_d = nc.dram_tensor("Wqq", [E, 128], F32R, kind="ExternalInput")
    Wkv_d = nc.dram_tensor("Wkv", [E, 128], F32R, kind="ExternalInput")
    bqq_d = nc.dram_tensor("bqq", [128, 1], F32, kind="ExternalInput")
    bkv_d = nc.dram_tensor("bkv", [128, 1], F32, kind="ExternalInput")
    outT_d = nc.dram_tensor("outT", [VW, T], BF16, kind="ExternalOutput")

    with tile.TileContext(nc) as tc, ExitStack() as ctx:
        const = ctx.enter_context(tc.tile_pool(name="const", bufs=1))
        acts = ctx.enter_context(tc.tile_pool(name="acts", bufs=1))

        # ---- constants / small weights ----
        wqq_sb = const.tile([128, NKT, 128], F32R)
        wkv_sb = const.tile([128, NKT, 128], F32R)
        for kt in range(NKT):
            nc.sync.dma_start(wqq_sb[:, kt, :],
                              Wqq_d[kt * 128:(kt + 1) * 128, :])
            nc.sync.dma_start(wkv_sb[:, kt, :],
                              Wkv_d[kt * 128:(kt + 1) * 128, :])
        bgT_sb = const.tile([128, NOC], F32)
        nc.sync.dma_start(bgT_sb[:], bgT_d[:, :])
        bqq_sb = const.tile([128, 1], F32)
        nc.sync.dma_start(bqq_sb[:], bqq_d[:, :])
        bkv_sb = const.tile([128, 1], F32)
        nc.sync.dma_start(bkv_sb[:], bkv_d[:, :])
        ident = const.tile([128, 128], F32)
        make_identity(nc, ident[:])
        # shifted-view indicator bank: mega[p, j] = 1 iff j == 62 + p//64;
        # chunk oc's indicator = mega[:, 62-2oc : 126-2oc] (f32 memset,
        # bitcast to f32r at the matmul: f32r memsets fail walrus codegen)
        mega = const.tile([128, 127], F32)
        nc.gpsimd.memset(mega[:], 0.0)
        nc.gpsimd.memset(mega[0:64, 62:63], 1.0)
        nc.gpsimd.memset(mega[64:128, 63:64], 1.0)

        # ---- big weights / inputs, DMA'd in compute order ----
        wgpool = ctx.enter_context(tc.tile_pool(name="wgp", bufs=1))
        wg_sb = [wgpool.tile([128, D * D], F32R, tag=f"wg{kt}",
                             name=f"wg{kt}") for kt in range(NKT)]
        for oc in range(NOC):
            for kt in range(NKT):
                nc.scalar.dma_start(
                    wg_sb[kt][:, oc * 128:(oc + 1) * 128],
                    WgT_d[kt * 128:(kt + 1) * 128, oc * 128:(oc + 1) * 128])

        xpool = ctx.enter_context(tc.tile_pool(name="xp", bufs=1))
        xt_sb = [[xpool.tile([128, TCW], F32R, tag=f"x{kt}_{tcn}",
                             name=f"x{kt}_{tcn}")
                  for tcn in range(NTC)] for kt in range(NKT)]
        for tcn in range(NTC):
            for kt in range(NKT):
                nc.sync.dma_start(
                    xt_sb[kt][tcn][:],
                    xT_d[kt * 128:(kt + 1) * 128,
                         tcn * TCW:(tcn + 1) * TCW])

        # ---- persistent activations ----
        qT2_sb = acts.tile([128, T], F32)        # [qT; qT]
        kT_sb = acts.tile([D, T], F32R)
        v_sb = acts.tile([128, T // 128, VW], BF16)
        nc.gpsimd.memset(v_sb[:, :, D:D + 1], 1.0)   # denominator column
        kvpool = ctx.enter_context(tc.tile_pool(name="kvp", bufs=2))
        # per-block qgT [64, 2048]: cols = (2*half + i)*512 + c, i.e. the
        # block's token chunks in order; pooled so only one block's qg is
        # live at a time
        qgpool = ctx.enter_context(tc.tile_pool(name="qgt", bufs=1))

        gpool = ctx.enter_context(tc.tile_pool(name="gp", bufs=2))
        ppool = ctx.enter_context(tc.tile_pool(name="pp", bufs=3))
        espool = ctx.enter_context(tc.tile_pool(name="es", bufs=9))
        outp = ctx.enter_context(tc.tile_pool(name="outp", bufs=1))

        pp = ctx.enter_context(tc.tile_pool(name="ps", bufs=4,
                                            space="PSUM"))
        pp_qg = ctx.enter_context(tc.tile_pool(name="pqg", bufs=1,
                                               space="PSUM"))

        NMT = N // 128   # m-tiles per batch
        NNC = N // TCW   # n chunks per batch

        es_tiles = {}
        qgT_of = {}

        def sc_item(b, mt, nch):
            def run():
                blk, pair = b // 2, b % 2
                mc0 = b * N + mt * 128
                qc0 = pair * 1024 + nch * TCW
                ps_ = pp.tile([128, TCW], F32, tag="pg", name="ps")
                nc.tensor.matmul(
                    ps_[:], kT_sb[:, mc0:mc0 + 128],
                    qgT_of[blk][:, qc0:qc0 + TCW],
                    start=True, stop=True)
                e_t = espool.tile([128, TCW], BF16, tag="es")
                nc.scalar.activation(e_t[:], ps_[:], AF.Exp)
                es_tiles[(b, mt, nch)] = e_t
            return run

        def av_item(b, nch):
            def run():
                nc0 = b * N + nch * TCW
                pav = pp.tile([VW, TCW], F32, tag="pg", name="pav")
                for mt in range(NMT):
                    nc.tensor.matmul(pav[:], v_sb[:, b * NMT + mt, :],
                                     es_tiles[(b, mt, nch)][:],
                                     start=(mt == 0), stop=(mt == NMT - 1))
                o_t = outp.tile([VW, TCW], BF16, tag="o", name="o_t")
                nc.vector.tensor_copy(o_t[:], pav[:])
                nc.sync.dma_start(outT_d[:, nc0:nc0 + TCW], o_t[:])
            return run

        def batch_items(b):
            # nch-grouped so only 8 es tiles are live at once (es ring=9)
            items = []
            for nch in range(NNC):
                for mt in range(NMT):
                    items.append(sc_item(b, mt, nch))
                items.append(av_item(b, nch))
            return items

        def project_qkv(tcn):
            """qT2 / kT / v for one 512-token chunk."""
            sl = slice(tcn * TCW, (tcn + 1) * TCW)
            pq = pp.tile([128, TCW], F32, tag="pg", name="pqq")
            for kt in range(NKT):
                nc.tensor.matmul(pq[:], wqq_sb[:, kt, :], xt_sb[kt][tcn][:],
                                 start=(kt == 0), stop=(kt == NKT - 1))
            nc.scalar.activation(qT2_sb[:, sl], pq[:], AF.Identity,
                                 bias=bqq_sb[:])
            pkv = pp.tile([128, TCW], F32, tag="pg", name="pkv")
            for kt in range(NKT):
                nc.tensor.matmul(pkv[:], wkv_sb[:, kt, :], xt_sb[kt][tcn][:],
                                 start=(kt == 0), stop=(kt == NKT - 1))
            kv_stage = kvpool.tile([128, TCW], F32, tag="kv", name="kvst")
            nc.scalar.activation(kv_stage[:], pkv[:], AF.Identity,
                                 bias=bkv_sb[:])
            nc.vector.tensor_copy(kT_sb[:, sl], kv_stage[0:64, :])
            for sub in range(TCW // 128):
                mtile = tcn * (TCW // 128) + sub
                ptr = pp.tile([128, D], F32, tag="pg", name="vtr")
                nc.tensor.transpose(
                    ptr[:], kv_stage[64:128, sub * 128:(sub + 1) * 128],
                    ident[64:128, 64:128])
                nc.vector.tensor_copy(v_sb[:, mtile, 0:D], ptr[:])

        # ---------- main loop ----------
        for blk in range(2):
            tcs = [blk * 4 + i for i in range(4)]
            queue = (batch_items(2 * blk - 2)
                     + batch_items(2 * blk - 1)) if blk > 0 else []
            qi = 0
            if blk == 0:
                project_qkv(0)
                project_qkv(1)

            qg_ps = pp_qg.tile([D, 4 * TCW], F32, tag="qg", name="qg_ps")
            prod_live = {}

            def issue_ind(oc):
                for half in range(2):
                    prod = prod_live.pop((oc, half))
                    for i in range(2):
                        c0 = (2 * half + i) * TCW
                        nc.tensor.matmul(
                            qg_ps[:, c0:c0 + TCW],
                            mega[:, 62 - 2 * oc:126 - 2 * oc].bitcast(F32R),
                            prod[:, i * TCW:(i + 1) * TCW],
                            start=(oc == 0), stop=(oc == NOC - 1))

            ind_pend = []
            for oc in range(NOC):
                # the half-1 multiplies of round 0 read qT2 of the block's
                # last two chunks -> both must be projected before them
                if blk == 0 and oc == 0:
                    project_qkv(2)
                    project_qkv(3)
                if blk == 0 and oc in (NOC - 2, NOC - 1):
                    project_qkv(4 + oc - (NOC - 2))  # tc4, tc5
                if blk == 1 and oc == 0:
                    project_qkv(6)
                    project_qkv(7)
                for half in range(2):
                    prod = ppool.tile([128, 2 * TCW], F32R, tag="prod",
                                      name=f"prod{half}")
                    for i in range(2):
                        tcn = tcs[half * 2 + i]
                        pg = pp.tile([128, TCW], F32, tag="pg",
                                     name=f"pg{half}{i}")
                        for kt in range(NKT):
                            nc.tensor.matmul(
                                pg[:], wg_sb[kt][:, oc * 128:(oc + 1) * 128],
                                xt_sb[kt][tcn][:],
                                start=(kt == 0), stop=(kt == NKT - 1))
                        gt = gpool.tile([128, TCW], F32, tag="g")
                        nc.scalar.activation(gt[:], pg[:], AF.Gelu,
                                             bias=bgT_sb[:, oc:oc + 1])
                        nc.vector.tensor_tensor(
                            prod[:, i * TCW:(i + 1) * TCW], gt[:],
                            qT2_sb[:, tcn * TCW:(tcn + 1) * TCW],
                            op=ALU.mult)
                    prod_live[(oc, half)] = prod
                ind_pend.append(oc)
                if len(ind_pend) > 1:
                    issue_ind(ind_pend.pop(0))
                while qi < len(queue) and oc >= 2 \
                        and qi <= ((oc - 2) * len(queue)) // (NOC - 3):
                    queue[qi]()
                    qi += 1
            while ind_pend:
                issue_ind(ind_pend.pop(0))
            while qi < len(queue):
                queue[qi]()
                qi += 1
            # drain the qg accumulator (aligned, partitions 0:64)
            qgT_of[blk] = qgpool.tile([D, 4 * TCW], F32R, tag="qgt",
                                      name=f"qgt{blk}")
            nc.vector.tensor_copy(qgT_of[blk][:], qg_ps[:])

        # tail: attention for the last block's batches
        for b in (2, 3):
            for it in batch_items(b):
                it()

    nc.compile()
    _cache["nc"] = nc
    return nc


def _make_in_maps(x, Wq, bq, Wk, bk, Wv, bv, Wg, bg):
    x = np.asarray(x, dtype=np.float32)
    xT = np.ascontiguousarray(x.reshape(T, E).T)
    # transposed-g column permutation: chunk oc, partition p ->
    # original column d*64 + e with d = p % 64, e = 2*oc + p // 64
    p = np.arange(128)
    perm = np.concatenate(
        [(p % 64) * 64 + 2 * oc + p // 64 for oc in range(NOC)])

    def dbl(w):
        return np.ascontiguousarray(
            np.concatenate([w, w], axis=-1).astype(np.float32))

    in_maps = []
    for h in range(M):
        c0 = h * D
        g0 = h * D * D
        Wg_h = np.asarray(Wg[:, g0:g0 + D * D], np.float32)
        bg_h = np.asarray(bg[g0:g0 + D * D], np.float32)
        in_maps.append(dict(
            xT=xT,
            WgT=np.ascontiguousarray(Wg_h[:, perm]),
            bgT=np.ascontiguousarray(
                bg_h[perm].reshape(NOC, 128).T.astype(np.float32)),
            Wqq=dbl(Wq[:, c0:c0 + D]),
            Wkv=np.ascontiguousarray(np.concatenate(
                [Wk[:, c0:c0 + D], Wv[:, c0:c0 + D]],
                axis=1).astype(np.float32)),
            bqq=dbl(bq[c0:c0 + D]).reshape(128, 1),
            bkv=np.concatenate([bk[c0:c0 + D], bv[c0:c0 + D]])
            .reshape(128, 1).astype(np.float32),
        ))
    return in_maps


def kernel(x, Wq, bq, Wk, bk, Wv, bv, Wg, bg):
    from concourse import bass_utils

    nc = _build()
    in_maps = _make_in_maps(x, Wq, bq, Wk, bk, Wv, bv, Wg, bg)
    global _last_in_maps
    _last_in_maps = in_maps
    res = bass_utils.run_bass_kernel_spmd(nc, in_maps, core_ids=list(range(M)))
    out = np.empty((B, N, H, D), dtype=np.float32)
    for h in range(M):
        oT = np.asarray(res.results[h]["outT"], np.float32)   # [65, T]
        o = (oT[:D] / oT[D:D + 1]).T                          # [T, 64]
        out[:, :, h, :] = o.reshape(B, N, D)
    return out.reshape(B, N, E)


# revision 39
# speedup vs baseline: 1.1483x; 1.0935x over previous
"""Fused OOQKV attention kernel for Trainium2 — transposed-g design.

Math (per head h, one head per core):
  g = gelu(x @ Wg_h + bg_h)            # [T, 64, 64] per-token transform
  q,k,v = x @ W{q,k,v}_h + b           # [T, 64]
  qg[t] = q[t] @ g[t]
  att = softmax(qg @ k^T)              # per batch, no scaling
  out_h = att @ v

g is computed TRANSPOSED: outputs on partitions, tokens on the free
axis. Chunk oc (0..31) holds the 128 outputs {(d, e): e in {2oc, 2oc+1}}
at partition p = (e - 2oc)*64 + d (host permutes Wg's columns). This
kills the three taxes the natural layout paid:
  - bg becomes a per-partition scalar fused into the ACT gelu (K=1 bias
    matmuls cost ~300ns each / ~86us of PE regardless of dtype)
  - qg[e] = sum_d q_d g[(d,e)] becomes a partition-group reduction: PE
    indicator matmuls (~35us) instead of ~184us of DVE reduce. All 32
    indicators are column-shifted views of one [128, 127] tile
    (mega[p, j] = 1 iff j == 62 + p//64).
  - gelu, multiply and reduce all touch contiguous memory; the natural
    layout forces an (e,d) interleave tax on exactly one engine
    (measured: gelu 1149 vs 683ns, DVE mult 967 vs 602ns per 512 elems)
qgT and kT come out pre-transposed for attention; only v transposes
back to natural (av's lhsT contracts over tokens).

q and k are each projected DOUBLED ([W|W] -> 128 rows) so partition p
reads q at d = p%64 for the DVE multiply, and so pair-1 scores read
kT/qgT at matching base-64 partitions (no cross-partition copies,
which no compute engine can do).

Schedule: 2 blocks of 4 token-chunks (2048 tokens = 2 batches each).
Wg[oc, kt] stays stationary across the block's 4 N=512 streams.
Software pipelining: gelu+mult of chunk oc issue one round late, the
indicator matmuls two rounds late, so the PE queue never head-of-line
blocks on the ACT->DVE chain. Attention for a block's two batches
interleaves into the next block's PE stream; batches 2,3 drain in a
tail. PSUM (16KB/partition, 2KB banks): 4-buf g ring + [128, 1024] qg
accumulator (pair0 rows 0:64, pair1 rows 64:128, each matmul writing a
2KB bank window) + 2-buf attention/qkv ring = exactly 8 banks.

All matmuls dtype-homogeneous (compiler rejects 32/16-bit mixing):
f32r except av in bf16 (v, es rounded to bf16: ~1e-3 output error at
2e-2 tolerance). The softmax denominator rides as a memset ones-column
of v; the host divides and transposes during the gather.
"""

import sys

sys.path.insert(0, "/opt/trn_rl_repo")

import numpy as np

B, N, E, H, D = 4, 1024, 512, 8, 64
T = B * N                 # 4096 flattened tokens
NKT = E // 128            # 4 contraction tiles
NOC = (D * D) // 128      # 32 transposed g chunks
TCW = 512                 # token chunk width
NTC = T // TCW            # 8 token chunks
VW = D + 1                # v | ones column
M = 8                     # cores

_cache = {}


def _build():
    if "nc" in _cache:
        return _cache["nc"]
    from contextlib import ExitStack

    import concourse.bass as bass
    import concourse.bacc as bacc
    import concourse.mybir as mybir
    import concourse.tile as tile
    from concourse.masks import make_identity

    F32 = mybir.dt.float32
    F32R = mybir.dt.float32r
    BF16 = mybir.dt.bfloat16
    AF = mybir.ActivationFunctionType
    ALU = mybir.AluOpType

    nc = bacc.Bacc(trn_type="TRN2")
    xT_d = nc.dram_tensor("xT", [E, T], F32R, kind="ExternalInput")
    WgT_d = nc.dram_tensor("WgT", [E, D * D], F32R, kind="ExternalInput")
    bgT_d = nc.dram_tensor("bgT", [128, NOC], F32, kind="ExternalInput")
    Wqq_d = nc.dram_tensor("Wqq", [E, 128], F32R, kind="ExternalInput")
    Wkv_d = nc.dram_tensor("Wkv", [E, 128], F32R, kind="ExternalInput")
    bqq_d = nc.dram_tensor("bqq", [128, 1], F32, kind="ExternalInput")
    bkv_d = nc.dram_tensor("bkv", [128, 1], F32, kind="ExternalInput")
    outT_d = nc.dram_tensor("outT", [VW, T], BF16, kind="ExternalOutput")

    with tile.TileContext(nc) as tc, ExitStack() as ctx:
        const = ctx.enter_context(tc.tile_pool(name="const", bufs=1))
        acts = ctx.enter_context(tc.tile_pool(name="acts", bufs=1))

        # ---- constants / small weights ----
        wqq_sb = const.tile([128, NKT, 128], F32R)
        wkv_sb = const.tile([128, NKT, 128], F32R)
        for kt in range(NKT):
            nc.sync.dma_start(wqq_sb[:, kt, :],
                              Wqq_d[kt * 128:(kt + 1) * 128, :])
            nc.sync.dma_start(wkv_sb[:, kt, :],
                              Wkv_d[kt * 128:(kt + 1) * 128, :])
        bgT_sb = const.tile([128, NOC], F32)
        nc.sync.dma_start(bgT_sb[:], bgT_d[:, :])
        bqq_sb = const.tile([128, 1], F32)
        nc.sync.dma_start(bqq_sb[:], bqq_d[:, :])
        bkv_sb = const.tile([128, 1], F32)
        nc.sync.dma_start(bkv_sb[:], bkv_d[:, :])
        ident = const.tile([128, 128], F32)
        make_identity(nc, ident[:])
        # shifted-view indicator bank: mega[p, j] = 1 iff j == 62 + p//64;
        # chunk oc's indicator = mega[:, 62-2oc : 126-2oc] (f32 memset,
        # bitcast to f32r at the matmul: f32r memsets fail walrus codegen)
        mega = const.tile([128, 127], F32)
        nc.gpsimd.memset(mega[:], 0.0)
        nc.gpsimd.memset(mega[0:64, 62:63], 1.0)
        nc.gpsimd.memset(mega[64:128, 63:64], 1.0)

        # ---- big weights / inputs, DMA'd in compute order ----
        wgpool = ctx.enter_context(tc.tile_pool(name="wgp", bufs=1))
        wg_sb = [wgpool.tile([128, D * D], F32R, tag=f"wg{kt}",
                             name=f"wg{kt}") for kt in range(NKT)]
        # DMA triggers ride the (otherwise idle) GpSimd sequencer — on the
        # ACT sequencer 128 DIRECT2D descriptors serialized ~81us ahead of
        # the gelus
        for oc in range(NOC):
            for kt in range(NKT):
                nc.gpsimd.dma_start(
                    wg_sb[kt][:, oc * 128:(oc + 1) * 128],
                    WgT_d[kt * 128:(kt + 1) * 128, oc * 128:(oc + 1) * 128])

        xpool = ctx.enter_context(tc.tile_pool(name="xp", bufs=1))
        xt_sb = [[xpool.tile([128, TCW], F32R, tag=f"x{kt}_{tcn}",
                             name=f"x{kt}_{tcn}")
                  for tcn in range(NTC)] for kt in range(NKT)]
        for tcn in range(NTC):
            for kt in range(NKT):
                nc.sync.dma_start(
                    xt_sb[kt][tcn][:],
                    xT_d[kt * 128:(kt + 1) * 128,
                         tcn * TCW:(tcn + 1) * TCW])

        # ---- persistent activations ----
        qT2_sb = acts.tile([128, T], F32)        # [qT; qT]
        kT_sb = acts.tile([D, T], F32R)
        v_sb = acts.tile([128, T // 128, VW], BF16)
        nc.gpsimd.memset(v_sb[:, :, D:D + 1], 1.0)   # denominator column
        kvpool = ctx.enter_context(tc.tile_pool(name="kvp", bufs=2))
        # per-block qgT [64, 2048]: cols = (2*half + i)*512 + c, i.e. the
        # block's token chunks in order; pooled so only one block's qg is
        # live at a time
        qgpool = ctx.enter_context(tc.tile_pool(name="qgt", bufs=1))

        gpool = ctx.enter_context(tc.tile_pool(name="gp", bufs=2))
        ppool = ctx.enter_context(tc.tile_pool(name="pp", bufs=3))
        espool = ctx.enter_context(tc.tile_pool(name="es", bufs=9))
        outp = ctx.enter_context(tc.tile_pool(name="outp", bufs=1))

        pp = ctx.enter_context(tc.tile_pool(name="ps", bufs=4,
                                            space="PSUM"))
        pp_qg = ctx.enter_context(tc.tile_pool(name="pqg", bufs=1,
                                               space="PSUM"))

        NMT = N // 128   # m-tiles per batch
        NNC = N // TCW   # n chunks per batch

        es_tiles = {}
        qgT_of = {}

        def sc_item(b, mt, nch):
            def run():
                blk, pair = b // 2, b % 2
                mc0 = b * N + mt * 128
                qc0 = pair * 1024 + nch * TCW
                ps_ = pp.tile([128, TCW], F32, tag="pg", name="ps")
                nc.tensor.matmul(
                    ps_[:], kT_sb[:, mc0:mc0 + 128],
                    qgT_of[blk][:, qc0:qc0 + TCW],
                    start=True, stop=True)
                e_t = espool.tile([128, TCW], BF16, tag="es")
                nc.scalar.activation(e_t[:], ps_[:], AF.Exp)
                es_tiles[(b, mt, nch)] = e_t
            return run

        def av_item(b, nch):
            def run():
                nc0 = b * N + nch * TCW
                pav = pp.tile([VW, TCW], F32, tag="pg", name="pav")
                for mt in range(NMT):
                    nc.tensor.matmul(pav[:], v_sb[:, b * NMT + mt, :],
                                     es_tiles[(b, mt, nch)][:],
                                     start=(mt == 0), stop=(mt == NMT - 1))
                o_t = outp.tile([VW, TCW], BF16, tag="o", name="o_t")
                nc.vector.tensor_copy(o_t[:], pav[:])
                nc.sync.dma_start(outT_d[:, nc0:nc0 + TCW], o_t[:])
            return run

        def batch_items(b):
            # nch-grouped so only 8 es tiles are live at once (es ring=9)
            items = []
            for nch in range(NNC):
                for mt in range(NMT):
                    items.append(sc_item(b, mt, nch))
                items.append(av_item(b, nch))
            return items

        def project_qkv(tcn):
            """qT2 / kT / v for one 512-token chunk."""
            sl = slice(tcn * TCW, (tcn + 1) * TCW)
            pq = pp.tile([128, TCW], F32, tag="pg", name="pqq")
            for kt in range(NKT):
                nc.tensor.matmul(pq[:], wqq_sb[:, kt, :], xt_sb[kt][tcn][:],
                                 start=(kt == 0), stop=(kt == NKT - 1))
            nc.scalar.activation(qT2_sb[:, sl], pq[:], AF.Identity,
                                 bias=bqq_sb[:])
            pkv = pp.tile([128, TCW], F32, tag="pg", name="pkv")
            for kt in range(NKT):
                nc.tensor.matmul(pkv[:], wkv_sb[:, kt, :], xt_sb[kt][tcn][:],
                                 start=(kt == 0), stop=(kt == NKT - 1))
            kv_stage = kvpool.tile([128, TCW], F32, tag="kv", name="kvst")
            nc.scalar.activation(kv_stage[:], pkv[:], AF.Identity,
                                 bias=bkv_sb[:])
            nc.vector.tensor_copy(kT_sb[:, sl], kv_stage[0:64, :])
            for sub in range(TCW // 128):
                mtile = tcn * (TCW // 128) + sub
                ptr = pp.tile([128, D], F32, tag="pg", name="vtr")
                nc.tensor.transpose(
                    ptr[:], kv_stage[64:128, sub * 128:(sub + 1) * 128],
                    ident[64:128, 64:128])
                nc.vector.tensor_copy(v_sb[:, mtile, 0:D], ptr[:])

        # ---------- main loop ----------
        for blk in range(2):
            tcs = [blk * 4 + i for i in range(4)]
            queue = (batch_items(2 * blk - 2)
                     + batch_items(2 * blk - 1)) if blk > 0 else []
            qi = 0
            if blk == 0:
                project_qkv(0)
                project_qkv(1)

            qg_ps = pp_qg.tile([D, 4 * TCW], F32, tag="qg", name="qg_ps")
            prod_live = {}

            def issue_ind(oc):
                for half in range(2):
                    prod = prod_live.pop((oc, half))
                    for i in range(2):
                        c0 = (2 * half + i) * TCW
                        nc.tensor.matmul(
                            qg_ps[:, c0:c0 + TCW],
                            mega[:, 62 - 2 * oc:126 - 2 * oc].bitcast(F32R),
                            prod[:, i * TCW:(i + 1) * TCW],
                            start=(oc == 0), stop=(oc == NOC - 1))

            ind_pend = []
            for oc in range(NOC):
                # the half-1 multiplies of round 0 read qT2 of the block's
                # last two chunks -> both must be projected before them
                if blk == 0 and oc == 0:
                    project_qkv(2)
                    project_qkv(3)
                if blk == 0 and oc in (NOC - 2, NOC - 1):
                    project_qkv(4 + oc - (NOC - 2))  # tc4, tc5
                if blk == 1 and oc == 0:
                    project_qkv(6)
                    project_qkv(7)
                for half in range(2):
                    prod = ppool.tile([128, 2 * TCW], F32R, tag="prod",
                                      name=f"prod{half}")
                    for i in range(2):
                        tcn = tcs[half * 2 + i]
                        pg = pp.tile([128, TCW], F32, tag="pg",
                                     name=f"pg{half}{i}")
                        for kt in range(NKT):
                            nc.tensor.matmul(
                                pg[:], wg_sb[kt][:, oc * 128:(oc + 1) * 128],
                                xt_sb[kt][tcn][:],
                                start=(kt == 0), stop=(kt == NKT - 1))
                        gt = gpool.tile([128, TCW], F32, tag="g")
                        nc.scalar.activation(gt[:], pg[:], AF.Gelu,
                                             bias=bgT_sb[:, oc:oc + 1])
                        nc.vector.tensor_tensor(
                            prod[:, i * TCW:(i + 1) * TCW], gt[:],
                            qT2_sb[:, tcn * TCW:(tcn + 1) * TCW],
                            op=ALU.mult)
                    prod_live[(oc, half)] = prod
                ind_pend.append(oc)
                if len(ind_pend) > 1:
                    issue_ind(ind_pend.pop(0))
                # attention in two whole-batch bursts per block: each burst
                # costs one Gelu->Exp->Gelu act-table round trip (1.3us per
                # load) instead of one per item
                if queue and oc in (6, 18):
                    for it in queue[qi:qi + 18]:
                        it()
                    qi += 18
            while ind_pend:
                issue_ind(ind_pend.pop(0))
            while qi < len(queue):
                queue[qi]()
                qi += 1
            # drain the qg accumulator (aligned, partitions 0:64)
            qgT_of[blk] = qgpool.tile([D, 4 * TCW], F32R, tag="qgt",
                                      name=f"qgt{blk}")
            nc.vector.tensor_copy(qgT_of[blk][:], qg_ps[:])

        # tail: attention for the last block's batches
        for b in (2, 3):
            for it in batch_items(b):
                it()

    nc.compile()
    _cache["nc"] = nc
    return nc


def _make_in_maps(x, Wq, bq, Wk, bk, Wv, bv, Wg, bg):
    x = np.asarray(x, dtype=np.float32)
    xT = np.ascontiguousarray(x.reshape(T, E).T)
    # transposed-g column permutation: chunk oc, partition p ->
    # original column d*64 + e with d = p % 64, e = 2*oc + p // 64
    p = np.arange(128)
    perm = np.concatenate(
        [(p % 64) * 64 + 2 * oc + p // 64 for oc in range(NOC)])

    def dbl(w):
        return np.ascontiguousarray(
            np.concatenate([w, w], axis=-1).astype(np.float32))

    in_maps = []
    for h in range(M):
        c0 = h * D
        g0 = h * D * D
        Wg_h = np.asarray(Wg[:, g0:g0 + D * D], np.float32)
        bg_h = np.asarray(bg[g0:g0 + D * D], np.float32)
        in_maps.append(dict(
            xT=xT,
            WgT=np.ascontiguousarray(Wg_h[:, perm]),
            bgT=np.ascontiguousarray(
                bg_h[perm].reshape(NOC, 128).T.astype(np.float32)),
            Wqq=dbl(Wq[:, c0:c0 + D]),
            Wkv=np.ascontiguousarray(np.concatenate(
                [Wk[:, c0:c0 + D], Wv[:, c0:c0 + D]],
                axis=1).astype(np.float32)),
            bqq=dbl(bq[c0:c0 + D]).reshape(128, 1),
            bkv=np.concatenate([bk[c0:c0 + D], bv[c0:c0 + D]])
            .reshape(128, 1).astype(np.float32),
        ))
    return in_maps


def kernel(x, Wq, bq, Wk, bk, Wv, bv, Wg, bg):
    from concourse import bass_utils

    nc = _build()
    in_maps = _make_in_maps(x, Wq, bq, Wk, bk, Wv, bv, Wg, bg)
    global _last_in_maps
    _last_in_maps = in_maps
    res = bass_utils.run_bass_kernel_spmd(nc, in_maps, core_ids=list(range(M)))
    out = np.empty((B, N, H, D), dtype=np.float32)
    for h in range(M):
        oT = np.asarray(res.results[h]["outT"], np.float32)   # [65, T]
        o = (oT[:D] / oT[D:D + 1]).T                          # [T, 64]
        out[:, :, h, :] = o.reshape(B, N, D)
    return out.reshape(B, N, E)


# revision 51
# speedup vs baseline: 1.1487x; 1.0003x over previous
"""Fused OOQKV attention kernel for Trainium2 — transposed-g design.

Math (per head h, one head per core):
  g = gelu(x @ Wg_h + bg_h)            # [T, 64, 64] per-token transform
  q,k,v = x @ W{q,k,v}_h + b           # [T, 64]
  qg[t] = q[t] @ g[t]
  att = softmax(qg @ k^T)              # per batch, no scaling
  out_h = att @ v

g is computed TRANSPOSED: outputs on partitions, tokens on the free
axis. Chunk oc (0..31) holds the 128 outputs {(d, e): e in {2oc, 2oc+1}}
at partition p = (e - 2oc)*64 + d (host permutes Wg's columns). This
kills the three taxes the natural layout paid:
  - bg becomes a per-partition scalar fused into the ACT gelu (K=1 bias
    matmuls cost ~300ns each / ~86us of PE regardless of dtype)
  - qg[e] = sum_d q_d g[(d,e)] becomes a partition-group reduction: PE
    indicator matmuls (~35us) instead of ~184us of DVE reduce. All 32
    indicators are column-shifted views of one [128, 127] tile
    (mega[p, j] = 1 iff j == 62 + p//64).
  - gelu, multiply and reduce all touch contiguous memory; the natural
    layout forces an (e,d) interleave tax on exactly one engine
    (measured: gelu 1149 vs 683ns, DVE mult 967 vs 602ns per 512 elems)
qgT and kT come out pre-transposed for attention; only v transposes
back to natural (av's lhsT contracts over tokens).

q is projected DOUBLED ([Wq|Wq] -> 128 rows) so partition p reads q at
d = p%64 for the DVE multiply; k and v share one [Wk|Wv] projection
(k rows 0:64 copy aligned to kT, v rows 64:128 feed the transposes —
no cross-partition copies anywhere, which no compute engine can do).

Schedule: 2 blocks of 4 token-chunks (2048 tokens = 2 batches each).
The indicator matmuls issue one round late so the PE queue never
head-of-line blocks on the gelu->mult (ACT->DVE) chain. Attention for
a block's two batches interleaves into the next block's PE stream as
two whole-batch bursts (one Gelu->Exp->Gelu act-table round trip each,
1.3us per table load); batches 2,3 drain in a tail. Wg's 128 DMA
triggers ride the idle GpSimd sequencer (on the ACT sequencer they
serialized ~81us ahead of the gelus). PSUM (16KB/partition, 2KB
banks): 4-buf shared ring (g chunks, projections, scores, av — every
tile's readers issue inline, so ring reuse is race-free) + the
[64, 2048] column-split qg accumulator, each indicator matmul writing
one 2KB bank window (matmul dst must start at partition 0 and a
start=True zero-region covers one bank) = exactly 8 banks.

All matmuls dtype-homogeneous (compiler rejects 32/16-bit mixing):
f32r except av in bf16 (v, es rounded to bf16: ~1e-3 output error at
2e-2 tolerance). The softmax denominator rides as a memset ones-column
of v; the host divides and transposes during the gather.
"""

import sys

sys.path.insert(0, "/opt/trn_rl_repo")

import numpy as np

B, N, E, H, D = 4, 1024, 512, 8, 64
T = B * N                 # 4096 flattened tokens
NKT = E // 128            # 4 contraction tiles
NOC = (D * D) // 128      # 32 transposed g chunks
TCW = 512                 # token chunk width
NTC = T // TCW            # 8 token chunks
VW = D + 1                # v | ones column
M = 8                     # cores

_cache = {}


def _build():
    if "nc" in _cache:
        return _cache["nc"]
    from contextlib import ExitStack

    import concourse.bass as bass
    import concourse.bacc as bacc
    import concourse.mybir as mybir
    import concourse.tile as tile
    from concourse.masks import make_identity

    F32 = mybir.dt.float32
    F32R = mybir.dt.float32r
    BF16 = mybir.dt.bfloat16
    AF = mybir.ActivationFunctionType
    ALU = mybir.AluOpType

    nc = bacc.Bacc(trn_type="TRN2")
    xT_d = nc.dram_tensor("xT", [E, T], F32R, kind="ExternalInput")
    WgT_d = nc.dram_tensor("WgT", [E, D * D], F32R, kind="ExternalInput")
    bgT_d = nc.dram_tensor("bgT", [128, NOC], F32, kind="ExternalInput")
    Wqq_d = nc.dram_tensor("Wqq", [E, 128], F32R, kind="ExternalInput")
    Wkv_d = nc.dram_tensor("Wkv", [E, 128], F32R, kind="ExternalInput")
    bqq_d = nc.dram_tensor("bqq", [128, 1], F32, kind="ExternalInput")
    bkv_d = nc.dram_tensor("bkv", [128, 1], F32, kind="ExternalInput")
    outT_d = nc.dram_tensor("outT", [VW, T], BF16, kind="ExternalOutput")

    with tile.TileContext(nc) as tc, ExitStack() as ctx:
        const = ctx.enter_context(tc.tile_pool(name="const", bufs=1))
        acts = ctx.enter_context(tc.tile_pool(name="acts", bufs=1))

        # ---- constants / small weights ----
        wqq_sb = const.tile([128, NKT, 128], F32R)
        wkv_sb = const.tile([128, NKT, 128], F32R)
        for kt in range(NKT):
            nc.sync.dma_start(wqq_sb[:, kt, :],
                              Wqq_d[kt * 128:(kt + 1) * 128, :])
            nc.sync.dma_start(wkv_sb[:, kt, :],
                              Wkv_d[kt * 128:(kt + 1) * 128, :])
        bgT_sb = const.tile([128, NOC], F32)
        nc.sync.dma_start(bgT_sb[:], bgT_d[:, :])
        bqq_sb = const.tile([128, 1], F32)
        nc.sync.dma_start(bqq_sb[:], bqq_d[:, :])
        bkv_sb = const.tile([128, 1], F32)
        nc.sync.dma_start(bkv_sb[:], bkv_d[:, :])
        ident = const.tile([128, 128], F32)
        make_identity(nc, ident[:])
        # shifted-view indicator bank: mega[p, j] = 1 iff j == 62 + p//64;
        # chunk oc's indicator = mega[:, 62-2oc : 126-2oc] (f32 memset,
        # bitcast to f32r at the matmul: f32r memsets fail walrus codegen)
        mega = const.tile([128, 127], F32)
        nc.gpsimd.memset(mega[:], 0.0)
        nc.gpsimd.memset(mega[0:64, 62:63], 1.0)
        nc.gpsimd.memset(mega[64:128, 63:64], 1.0)

        # ---- big weights / inputs, DMA'd in compute order ----
        wgpool = ctx.enter_context(tc.tile_pool(name="wgp", bufs=1))
        wg_sb = [wgpool.tile([128, D * D], F32R, tag=f"wg{kt}",
                             name=f"wg{kt}") for kt in range(NKT)]
        # DMA triggers ride the (otherwise idle) GpSimd sequencer — on the
        # ACT sequencer 128 DIRECT2D descriptors serialized ~81us ahead of
        # the gelus
        for oc in range(NOC):
            for kt in range(NKT):
                nc.gpsimd.dma_start(
                    wg_sb[kt][:, oc * 128:(oc + 1) * 128],
                    WgT_d[kt * 128:(kt + 1) * 128, oc * 128:(oc + 1) * 128])

        xpool = ctx.enter_context(tc.tile_pool(name="xp", bufs=1))
        xt_sb = [[xpool.tile([128, TCW], F32R, tag=f"x{kt}_{tcn}",
                             name=f"x{kt}_{tcn}")
                  for tcn in range(NTC)] for kt in range(NKT)]
        for tcn in range(NTC):
            for kt in range(NKT):
                nc.sync.dma_start(
                    xt_sb[kt][tcn][:],
                    xT_d[kt * 128:(kt + 1) * 128,
                         tcn * TCW:(tcn + 1) * TCW])

        # ---- persistent activations ----
        qT2_sb = acts.tile([128, T], F32)        # [qT; qT]
        kT_sb = acts.tile([D, T], F32R)
        v_sb = acts.tile([128, T // 128, VW], BF16)
        nc.gpsimd.memset(v_sb[:, :, D:D + 1], 1.0)   # denominator column
        kvpool = ctx.enter_context(tc.tile_pool(name="kvp", bufs=2))
        # per-block qgT [64, 2048]: cols = block's token chunks in order;
        # pooled (bufs=1) so only one block's qg is live at a time — the
        # previous block's tile is reused once its scores have read it
        qgpool = ctx.enter_context(tc.tile_pool(name="qgt", bufs=1))

        gpool = ctx.enter_context(tc.tile_pool(name="gp", bufs=2))
        ppool = ctx.enter_context(tc.tile_pool(name="pp", bufs=3))
        espool = ctx.enter_context(tc.tile_pool(name="es", bufs=9))
        outp = ctx.enter_context(tc.tile_pool(name="outp", bufs=1))

        pp = ctx.enter_context(tc.tile_pool(name="ps", bufs=4,
                                            space="PSUM"))
        pp_qg = ctx.enter_context(tc.tile_pool(name="pqg", bufs=1,
                                               space="PSUM"))

        NMT = N // 128   # m-tiles per batch
        NNC = N // TCW   # n chunks per batch

        es_tiles = {}
        qgT_of = {}

        def sc_item(b, mt, nch):
            def run():
                blk, pair = b // 2, b % 2
                mc0 = b * N + mt * 128
                qc0 = pair * 1024 + nch * TCW
                ps_ = pp.tile([128, TCW], F32, tag="pg", name="ps")
                nc.tensor.matmul(
                    ps_[:], kT_sb[:, mc0:mc0 + 128],
                    qgT_of[blk][:, qc0:qc0 + TCW],
                    start=True, stop=True)
                e_t = espool.tile([128, TCW], BF16, tag="es")
                nc.scalar.activation(e_t[:], ps_[:], AF.Exp)
                es_tiles[(b, mt, nch)] = e_t
            return run

        def av_item(b, nch):
            def run():
                nc0 = b * N + nch * TCW
                pav = pp.tile([VW, TCW], F32, tag="pg", name="pav")
                for mt in range(NMT):
                    nc.tensor.matmul(pav[:], v_sb[:, b * NMT + mt, :],
                                     es_tiles[(b, mt, nch)][:],
                                     start=(mt == 0), stop=(mt == NMT - 1))
                o_t = outp.tile([VW, TCW], BF16, tag="o", name="o_t")
                nc.vector.tensor_copy(o_t[:], pav[:])
                nc.sync.dma_start(outT_d[:, nc0:nc0 + TCW], o_t[:])
            return run

        def batch_items(b):
            # nch-grouped so only 8 es tiles are live at once (es ring=9)
            items = []
            for nch in range(NNC):
                for mt in range(NMT):
                    items.append(sc_item(b, mt, nch))
                items.append(av_item(b, nch))
            return items

        def project_qkv(tcn):
            """qT2 / kT / v for one 512-token chunk."""
            sl = slice(tcn * TCW, (tcn + 1) * TCW)
            pq = pp.tile([128, TCW], F32, tag="pg", name="pqq")
            for kt in range(NKT):
                nc.tensor.matmul(pq[:], wqq_sb[:, kt, :], xt_sb[kt][tcn][:],
                                 start=(kt == 0), stop=(kt == NKT - 1))
            nc.scalar.activation(qT2_sb[:, sl], pq[:], AF.Identity,
                                 bias=bqq_sb[:])
            pkv = pp.tile([128, TCW], F32, tag="pg", name="pkv")
            for kt in range(NKT):
                nc.tensor.matmul(pkv[:], wkv_sb[:, kt, :], xt_sb[kt][tcn][:],
                                 start=(kt == 0), stop=(kt == NKT - 1))
            kv_stage = kvpool.tile([128, TCW], F32, tag="kv", name="kvst")
            nc.scalar.activation(kv_stage[:], pkv[:], AF.Identity,
                                 bias=bkv_sb[:])
            nc.vector.tensor_copy(kT_sb[:, sl], kv_stage[0:64, :])
            for sub in range(TCW // 128):
                mtile = tcn * (TCW // 128) + sub
                ptr = pp.tile([128, D], F32, tag="pg", name="vtr")
                nc.tensor.transpose(
                    ptr[:], kv_stage[64:128, sub * 128:(sub + 1) * 128],
                    ident[64:128, 64:128])
                nc.vector.tensor_copy(v_sb[:, mtile, 0:D], ptr[:])

        # ---------- main loop ----------
        for blk in range(2):
            tcs = [blk * 4 + i for i in range(4)]
            queue = (batch_items(2 * blk - 2)
                     + batch_items(2 * blk - 1)) if blk > 0 else []
            qi = 0
            if blk == 0:
                project_qkv(0)
                project_qkv(1)

            qg_ps = pp_qg.tile([D, 4 * TCW], F32, tag="qg", name="qg_ps")
            prod_live = {}

            def issue_ind(oc):
                for half in range(2):
                    prod = prod_live.pop((oc, half))
                    for i in range(2):
                        c0 = (2 * half + i) * TCW
                        nc.tensor.matmul(
                            qg_ps[:, c0:c0 + TCW],
                            mega[:, 62 - 2 * oc:126 - 2 * oc].bitcast(F32R),
                            prod[:, i * TCW:(i + 1) * TCW],
                            start=(oc == 0), stop=(oc == NOC - 1))

            ind_pend = []
            for oc in range(NOC):
                # the half-1 multiplies of round 0 read qT2 of the block's
                # last two chunks -> both must be projected before them
                if blk == 0 and oc == 0:
                    project_qkv(2)
                    project_qkv(3)
                if blk == 0 and oc in (NOC - 2, NOC - 1):
                    project_qkv(4 + oc - (NOC - 2))  # tc4, tc5
                if blk == 1 and oc == 0:
                    project_qkv(6)
                    project_qkv(7)
                for half in range(2):
                    prod = ppool.tile([128, 2 * TCW], F32R, tag="prod",
                                      name=f"prod{half}")
                    for i in range(2):
                        tcn = tcs[half * 2 + i]
                        pg = pp.tile([128, TCW], F32, tag="pg",
                                     name=f"pg{half}{i}")
                        for kt in range(NKT):
                            nc.tensor.matmul(
                                pg[:], wg_sb[kt][:, oc * 128:(oc + 1) * 128],
                                xt_sb[kt][tcn][:],
                                start=(kt == 0), stop=(kt == NKT - 1))
                        gt = gpool.tile([128, TCW], F32, tag="g")
                        nc.scalar.activation(gt[:], pg[:], AF.Gelu,
                                             bias=bgT_sb[:, oc:oc + 1])
                        nc.vector.tensor_tensor(
                            prod[:, i * TCW:(i + 1) * TCW], gt[:],
                            qT2_sb[:, tcn * TCW:(tcn + 1) * TCW],
                            op=ALU.mult)
                    prod_live[(oc, half)] = prod
                ind_pend.append(oc)
                if len(ind_pend) > 1:
                    issue_ind(ind_pend.pop(0))
                # attention in two whole-batch bursts per block: each burst
                # costs one Gelu->Exp->Gelu act-table round trip (1.3us per
                # load) instead of one per item
                if queue and oc in (6, 18):
                    for it in queue[qi:qi + 18]:
                        it()
                    qi += 18
            while ind_pend:
                issue_ind(ind_pend.pop(0))
            while qi < len(queue):
                queue[qi]()
                qi += 1
            # drain the qg accumulator (aligned, partitions 0:64)
            qgT_of[blk] = qgpool.tile([D, 4 * TCW], F32R, tag="qgt",
                                      name=f"qgt{blk}")
            nc.vector.tensor_copy(qgT_of[blk][:], qg_ps[:])

        # tail: attention for the last block's batches
        for b in (2, 3):
            for it in batch_items(b):
                it()

    nc.compile()
    _cache["nc"] = nc
    return nc


def _make_in_maps(x, Wq, bq, Wk, bk, Wv, bv, Wg, bg):
    x = np.asarray(x, dtype=np.float32)
    xT = np.ascontiguousarray(x.reshape(T, E).T)
    # transposed-g column permutation: chunk oc, partition p ->
    # original column d*64 + e with d = p % 64, e = 2*oc + p // 64
    p = np.arange(128)
    perm = np.concatenate(
        [(p % 64) * 64 + 2 * oc + p // 64 for oc in range(NOC)])

    def dbl(w):
        return np.ascontiguousarray(
            np.concatenate([w, w], axis=-1).astype(np.float32))

    in_maps = []
    for h in range(M):
        c0 = h * D
        g0 = h * D * D
        Wg_h = np.asarray(Wg[:, g0:g0 + D * D], np.float32)
        bg_h = np.asarray(bg[g0:g0 + D * D], np.float32)
        in_maps.append(dict(
            xT=xT,
            WgT=np.ascontiguousarray(Wg_h[:, perm]),
            bgT=np.ascontiguousarray(
                bg_h[perm].reshape(NOC, 128).T.astype(np.float32)),
            Wqq=dbl(Wq[:, c0:c0 + D]),
            Wkv=np.ascontiguousarray(np.concatenate(
                [Wk[:, c0:c0 + D], Wv[:, c0:c0 + D]],
                axis=1).astype(np.float32)),
            bqq=dbl(bq[c0:c0 + D]).reshape(128, 1),
            bkv=np.concatenate([bk[c0:c0 + D], bv[c0:c0 + D]])
            .reshape(128, 1).astype(np.float32),
        ))
    return in_maps


def kernel(x, Wq, bq, Wk, bk, Wv, bv, Wg, bg):
    from concourse import bass_utils

    nc = _build()
    in_maps = _make_in_maps(x, Wq, bq, Wk, bk, Wv, bv, Wg, bg)
    global _last_in_maps
    _last_in_maps = in_maps
    res = bass_utils.run_bass_kernel_spmd(nc, in_maps, core_ids=list(range(M)))
    out = np.empty((B, N, H, D), dtype=np.float32)
    for h in range(M):
        oT = np.asarray(res.results[h]["outT"], np.float32)   # [65, T]
        o = (oT[:D] / oT[D:D + 1]).T                          # [T, 64]
        out[:, :, h, :] = o.reshape(B, N, D)
    return out.reshape(B, N, E)
